# revision 3
# baseline (speedup 1.0000x reference)
"""Trainium2 Bass kernel for nn_Conv_27693949125154.

Each 128-dim vector is a 16x8 image; valid 3x3 conv with the fixed kernel
[[1,0,1],[0,1,0],[1,0,1]] then relu -> 84 outputs (14x6).

All five stencil taps live inside each letter's own 128-element block
(flat pixel index k = i*8 + j): out_k = x[k] + x[k+2] + x[k+16] + x[k+18]
+ x[k+9].  That makes the first two partial sums expressible as *fully
contiguous* shifted adds over the whole chunk (full-rate DVE), with only
the final tap-gather op strided:

    h[k] = x[k] + x[k+2]        contiguous, 128*t-2 elems
    p[k] = h[k] + h[k+16]       contiguous, 128*t-18 elems
    s[t,i,j] = p[t,128t+8i+j] + x[t,128t+8i+j+9]   (strided, 84/letter)
    y = relu(s)                 ACT engine, contiguous, + store

The whole pipeline runs in bf16 (the conv is 4 adds; measured end-to-end
rel err ~7e-3, well inside the 2e-2 gate), which halves both HBM traffic
(12.2 MB/core -> ~34 us roofline at 358 GB/s) and DVE element time.
The host casts f32->bf16 before upload and back after.

Layout: letters (B*W rows) on SBUF partitions, pixels along the free
dim.  Input loads double-ring (sync + scalar HWDGE) with small leading
reads for pipeline ramp; relu'd outputs stream out on the scalar ring.

Pure data parallel over 8 NeuronCores (batch sharding, no comm).
"""

import numpy as np
import ml_dtypes

import concourse.bass as bass
import concourse.mybir as mybir
from concourse import tile
from concourse.bass_utils import run_bass_kernel_spmd

# Full problem: x (16384, 14, 128) f32 -> out (16384, 14, 84) f32
B, W, L = 16384, 14, 128
OUT = 84
N_CORES = 8
ROWS = B * W                     # 229376 letters total
ROWS_PER_CORE = ROWS // N_CORES  # 28672
P = 128                          # SBUF partitions

BF16 = mybir.dt.bfloat16
NP_BF16 = ml_dtypes.bfloat16


def split_multi_waits(nc, max_waits=1):
    """walrus CoreV3 codegen rejects instructions with several sync-wait
    conditions; hoist extras onto NOPs inserted just before, same engine."""
    for f in nc.m.functions:
        for blk in f.blocks:
            new = []
            for inst in blk.instructions:
                si = inst.sync_info
                if si is not None and si.on_wait and len(si.on_wait) > max_waits:
                    waits = list(si.on_wait)
                    head, tail = waits[:-max_waits], waits[-max_waits:]
                    for k, w in enumerate(head):
                        new.append(
                            mybir.InstNoOp(
                                name=f"{inst.name}-wsplit{k}",
                                engine=inst.engine,
                                ins=[],
                                outs=[],
                                sync_info=mybir.SyncInfo(on_wait=[w], on_update=[]),
                            )
                        )
                    inst.sync_info = mybir.SyncInfo(
                        on_wait=tail, on_update=list(si.on_update)
                    )
                new.append(inst)
            blk.instructions = new


def build_program(rows=ROWS_PER_CORE, read_sizes=None, chunk_sizes=None,
                  split_waits=True, o_bufs=2, work_bufs=2, r_bufs=2,
                  s_engine="vector"):
    """Per-core program: x [rows,128] bf16 -> y [rows,84] bf16."""
    t_total = rows // P                  # letters per partition (224)
    if read_sizes is None:
        read_sizes = [4, 4, 6, 14, 14, 14] + [28] * ((t_total - 56) // 28)
    if chunk_sizes is None:
        chunk_sizes = [7, 14, 42, 42, 42, 42, 21, 7, 7]
    assert sum(read_sizes) == t_total and sum(chunk_sizes) == t_total
    t_c_max = max(chunk_sizes)

    nc = bass.Bass(
        "TRN2", target_bir_lowering=False, debug=False, num_devices=N_CORES
    )
    x = nc.dram_tensor("x", [rows, L], BF16, kind="ExternalInput")
    y = nc.dram_tensor("y", [rows, OUT], BF16, kind="ExternalOutput")

    # partition p holds letters [p*t_total, (p+1)*t_total)
    xf = x.ap().rearrange("(p t) m -> p (t m)", p=P)   # [P, t_total*128]
    yf = y.ap().rearrange("(p t) m -> p (t m)", p=P)   # [P, t_total*84]

    with tile.TileContext(nc) as tc:
        with (
            tc.tile_pool(name="xin", bufs=1) as xin_pool,
            tc.tile_pool(name="oout", bufs=o_bufs) as oout_pool,
            tc.tile_pool(name="hpool", bufs=r_bufs) as hpool,
            tc.tile_pool(name="ppool", bufs=r_bufs) as ppool,
            tc.tile_pool(name="spool", bufs=work_bufs) as spool,
        ):
            xt = xin_pool.tile([P, t_total * L], BF16, tag="x")
            # all reads upfront into disjoint slices -> max read-ahead
            off = 0
            for k, sz in enumerate(read_sizes):
                eng = nc.scalar if (k % 2 == 1 and k < 8) else nc.sync
                eng.dma_start(
                    out=xt[:, off * L : (off + sz) * L],
                    in_=xf[:, off * L : (off + sz) * L],
                )
                off += sz

            X4 = xt.rearrange("p (t i j) -> p t i j", i=16, j=8)  # [P,t,16,8]
            s_eng = nc.gpsimd if s_engine == "gpsimd" else nc.vector
            off = 0
            for t_c in chunk_sizes:
                x0 = off * L                       # chunk base, flat elems
                n1 = t_c * L - 2
                n2 = t_c * L - 18

                # h[k] = x[k] + x[k+2], contiguous full-rate
                ht = hpool.tile([P, t_c_max * L], BF16, tag="h", name="h")
                nc.vector.tensor_tensor(
                    ht[:, :n1], xt[:, x0 : x0 + n1], xt[:, x0 + 2 : x0 + 2 + n1],
                    mybir.AluOpType.add,
                )

                # p[k] = h[k] + h[k+16], contiguous full-rate
                pt = ppool.tile([P, t_c_max * L], BF16, tag="p", name="p")
                nc.vector.tensor_tensor(
                    pt[:, :n2], ht[:, :n2], ht[:, 16 : 16 + n2],
                    mybir.AluOpType.add,
                )

                # s = p(taps) + x(center taps), strided gather -> compact 84
                st = spool.tile([P, t_c_max * 84], BF16, tag="s", name="s")
                s4 = st.rearrange("p (t i j) -> p t i j", i=14, j=6)[:, :t_c]
                p4 = pt.rearrange("p (t i j) -> p t i j", i=16, j=8)
                s_eng.tensor_tensor(
                    s4, p4[:, :t_c, 0:14, 0:6], X4[:, off : off + t_c, 1:15, 1:7],
                    mybir.AluOpType.add,
                )

                # relu on the scalar engine; out-DMA on the scalar ring
                ot = oout_pool.tile([P, t_c_max * OUT], BF16, tag="o", name="ot")[
                    :, : t_c * OUT
                ]
                nc.scalar.activation(
                    ot[:], st[:, : t_c * 84], mybir.ActivationFunctionType.Relu
                )
                nc.scalar.dma_start(
                    out=yf[:, off * OUT : (off + t_c) * OUT], in_=ot[:]
                )
                off += t_c

    if split_waits:
        split_multi_waits(nc)
    return nc


_nc_cache = {}


def _get_program():
    if "nc" not in _nc_cache:
        _nc_cache["nc"] = build_program(s_engine="gpsimd")
    return _nc_cache["nc"]


def make_in_maps(x):
    """Full f32 x (B,W,L) -> per-core bf16 in_maps."""
    xb = np.ascontiguousarray(x).astype(NP_BF16)
    shards = xb.reshape(N_CORES, ROWS_PER_CORE, L)
    return [{"x": shards[i]} for i in range(N_CORES)]


def kernel(x):
    x = np.asarray(x, dtype=np.float32)
    assert x.shape == (B, W, L), x.shape

    nc = _get_program()
    in_maps = make_in_maps(x)
    res = run_bass_kernel_spmd(nc, in_maps, core_ids=list(range(N_CORES)))
    out = np.concatenate(
        [
            np.asarray(res.results[i]["y"]).reshape(-1, W, OUT)
            for i in range(N_CORES)
        ],
        axis=0,
    )
    return out.astype(np.float32)


# revision 6
# speedup vs baseline: 1.3002x; 1.3002x over previous
"""Trainium2 Bass kernel for nn_Conv_27693949125154.

Each 128-dim vector is a 16x8 image; valid 3x3 conv with the fixed kernel
[[1,0,1],[0,1,0],[1,0,1]] then relu -> 84 outputs (14x6).

All five stencil taps live inside each letter's own 128-element block
(flat pixel index k = i*8 + j): out_k = x[k] + x[k+2] + x[k+16] + x[k+18]
+ x[k+9].  That makes the first two partial sums expressible as *fully
contiguous* shifted adds over the whole chunk (full-rate DVE), with only
the final tap-gather op strided:

    h[k] = x[k] + x[k+2]        contiguous, 128*t-2 elems
    p[k] = h[k] + h[k+16]       contiguous, 128*t-18 elems
    s[t,i,j] = p[t,128t+8i+j] + x[t,128t+8i+j+9]   (strided, 84/letter)
    y = relu(s)                 ACT engine, contiguous, + store

The whole pipeline runs in bf16 (the conv is 4 adds; measured end-to-end
rel err ~7e-3, well inside the 2e-2 gate), which halves both HBM traffic
(12.2 MB/core -> ~34 us roofline at 358 GB/s) and DVE element time.
The host casts f32->bf16 before upload and back after.

Layout: letters (B*W rows) on SBUF partitions, pixels along the free
dim.  Input loads double-ring (sync + scalar HWDGE) with small leading
reads for pipeline ramp; relu'd outputs stream out on the scalar ring.

Pure data parallel over 8 NeuronCores (batch sharding, no comm).
"""

import numpy as np
import ml_dtypes

import concourse.bass as bass
import concourse.mybir as mybir
from concourse import tile
from concourse.bass_utils import run_bass_kernel_spmd

# Full problem: x (16384, 14, 128) f32 -> out (16384, 14, 84) f32
B, W, L = 16384, 14, 128
OUT = 84
N_CORES = 8
ROWS = B * W                     # 229376 letters total
ROWS_PER_CORE = ROWS // N_CORES  # 28672
P = 128                          # SBUF partitions

BF16 = mybir.dt.bfloat16
NP_BF16 = ml_dtypes.bfloat16


def split_multi_waits(nc, max_waits=1):
    """walrus CoreV3 codegen rejects instructions with several sync-wait
    conditions; hoist extras onto NOPs inserted just before, same engine."""
    for f in nc.m.functions:
        for blk in f.blocks:
            new = []
            for inst in blk.instructions:
                si = inst.sync_info
                if si is not None and si.on_wait and len(si.on_wait) > max_waits:
                    waits = list(si.on_wait)
                    head, tail = waits[:-max_waits], waits[-max_waits:]
                    for k, w in enumerate(head):
                        new.append(
                            mybir.InstNoOp(
                                name=f"{inst.name}-wsplit{k}",
                                engine=inst.engine,
                                ins=[],
                                outs=[],
                                sync_info=mybir.SyncInfo(on_wait=[w], on_update=[]),
                            )
                        )
                    inst.sync_info = mybir.SyncInfo(
                        on_wait=tail, on_update=list(si.on_update)
                    )
                new.append(inst)
            blk.instructions = new


def build_program(rows=ROWS_PER_CORE, read_sizes=None, chunk_sizes=None,
                  split_waits=True, o_bufs=2, work_bufs=2, r_bufs=2,
                  s_engine="vector"):
    """Per-core program: x [rows,128] bf16 -> y [rows,84] bf16."""
    t_total = rows // P                  # letters per partition (224)
    if read_sizes is None:
        read_sizes = [4, 4, 6, 14, 14, 14] + [28] * ((t_total - 56) // 28)
    if chunk_sizes is None:
        chunk_sizes = [7, 14, 42, 42, 42, 42, 21, 7, 7]
    assert sum(read_sizes) == t_total and sum(chunk_sizes) == t_total
    t_c_max = max(chunk_sizes)

    nc = bass.Bass(
        "TRN2", target_bir_lowering=False, debug=False, num_devices=N_CORES
    )
    x = nc.dram_tensor("x", [rows, L], BF16, kind="ExternalInput")
    y = nc.dram_tensor("y", [rows, OUT], BF16, kind="ExternalOutput")

    # partition p holds letters [p*t_total, (p+1)*t_total)
    xf = x.ap().rearrange("(p t) m -> p (t m)", p=P)   # [P, t_total*128]
    yf = y.ap().rearrange("(p t) m -> p (t m)", p=P)   # [P, t_total*84]

    with tile.TileContext(nc) as tc:
        with (
            tc.tile_pool(name="xin", bufs=1) as xin_pool,
            tc.tile_pool(name="oout", bufs=o_bufs) as oout_pool,
            tc.tile_pool(name="hpool", bufs=r_bufs) as hpool,
            tc.tile_pool(name="ppool", bufs=r_bufs) as ppool,
            tc.tile_pool(name="spool", bufs=work_bufs) as spool,
        ):
            xt = xin_pool.tile([P, t_total * L], BF16, tag="x")
            # all reads upfront into disjoint slices -> max read-ahead
            off = 0
            for k, sz in enumerate(read_sizes):
                eng = nc.scalar if (k % 2 == 1 and k < 8) else nc.sync
                eng.dma_start(
                    out=xt[:, off * L : (off + sz) * L],
                    in_=xf[:, off * L : (off + sz) * L],
                )
                off += sz

            X4 = xt.rearrange("p (t i j) -> p t i j", i=16, j=8)  # [P,t,16,8]
            s_eng = nc.gpsimd if s_engine == "gpsimd" else nc.vector
            off = 0
            for t_c in chunk_sizes:
                x0 = off * L                       # chunk base, flat elems
                n1 = t_c * L - 2
                n2 = t_c * L - 18

                # h[k] = x[k] + x[k+2], contiguous full-rate
                ht = hpool.tile([P, t_c_max * L], BF16, tag="h", name="h")
                nc.vector.tensor_tensor(
                    ht[:, :n1], xt[:, x0 : x0 + n1], xt[:, x0 + 2 : x0 + 2 + n1],
                    mybir.AluOpType.add,
                )

                # p[k] = h[k] + h[k+16], contiguous full-rate
                pt = ppool.tile([P, t_c_max * L], BF16, tag="p", name="p")
                nc.vector.tensor_tensor(
                    pt[:, :n2], ht[:, :n2], ht[:, 16 : 16 + n2],
                    mybir.AluOpType.add,
                )

                # s = p(taps) + x(center taps), strided gather -> compact 84
                st = spool.tile([P, t_c_max * 84], BF16, tag="s", name="s")
                s4 = st.rearrange("p (t i j) -> p t i j", i=14, j=6)[:, :t_c]
                p4 = pt.rearrange("p (t i j) -> p t i j", i=16, j=8)
                s_eng.tensor_tensor(
                    s4, p4[:, :t_c, 0:14, 0:6], X4[:, off : off + t_c, 1:15, 1:7],
                    mybir.AluOpType.add,
                )

                # relu on the scalar engine; out-DMA on the scalar ring
                ot = oout_pool.tile([P, t_c_max * OUT], BF16, tag="o", name="ot")[
                    :, : t_c * OUT
                ]
                nc.scalar.activation(
                    ot[:], st[:, : t_c * 84], mybir.ActivationFunctionType.Relu
                )
                nc.scalar.dma_start(
                    out=yf[:, off * OUT : (off + t_c) * OUT], in_=ot[:]
                )
                off += t_c

    if split_waits:
        split_multi_waits(nc)
    return nc


N_LET = ROWS_PER_CORE                 # letters per core (28672)
MM = 512                              # letters per matmul (PSUM bank = 512 f32)
EV = 2048                             # letters per evict / store chunk (4 banks)


def conv_matrix():
    """[128, 84] 0/1 tap matrix: out[:, o] = sum of 5 taps of the letter."""
    m = np.zeros((L, OUT), dtype=np.float32)
    for i in range(14):
        for j in range(6):
            o = i * 6 + j
            for di, dj in ((0, 0), (0, 2), (1, 1), (2, 0), (2, 2)):
                m[(i + di) * 8 + (j + dj), o] = 1.0
    return m.astype(NP_BF16)


def build_program_pe(n=N_LET, read_sizes=None, ev_engine="vector"):
    """PE-matmul program: xT [128, n] bf16 -> y [84, n] bf16.

    Host supplies x transposed (pixels on partitions).  One matmul per 512
    letters against the 0/1 tap matrix does all 4 adds with exact f32 PSUM
    accumulation; relu + bf16 cast fused into the PSUM evict on DVE.
    """
    if read_sizes is None:
        read_sizes = [512, 512, 1024, 2048, 2048, 2048] + [4096] * 5
    assert sum(read_sizes) == n and n % EV == 0 and EV % MM == 0

    nc = bass.Bass(
        "TRN2", target_bir_lowering=False, debug=False, num_devices=N_CORES
    )
    x = nc.dram_tensor("x", [L, n], BF16, kind="ExternalInput")
    m = nc.dram_tensor("m", [L, OUT], BF16, kind="ExternalInput")
    y = nc.dram_tensor("y", [OUT, n], BF16, kind="ExternalOutput")

    with tile.TileContext(nc) as tc:
        with (
            tc.tile_pool(name="xin", bufs=1) as xin_pool,
            tc.tile_pool(name="mw", bufs=1) as mw_pool,
            tc.tile_pool(name="stage", bufs=2) as stage_pool,
            tc.psum_pool(name="acc", bufs=2) as acc_pool,
        ):
            mt = mw_pool.tile([L, OUT], BF16, tag="m")
            nc.sync.dma_start(out=mt[:], in_=m.ap())

            xt = xin_pool.tile([L, n], BF16, tag="x")
            off = 0
            for k, sz in enumerate(read_sizes):
                eng = nc.scalar if (k % 2 == 1 and k < 6) else nc.sync
                eng.dma_start(out=xt[:, off : off + sz], in_=x.ap()[:, off : off + sz])
                off += sz

            ev_eng = nc.vector if ev_engine == "vector" else nc.gpsimd
            for g in range(n // EV):
                acc = acc_pool.tile(
                    [OUT, EV // MM, MM], mybir.dt.float32, tag="acc", name="acc"
                )
                for q in range(EV // MM):
                    a = g * EV + q * MM
                    nc.tensor.matmul(
                        acc[:, q], mt[:], xt[:, a : a + MM], start=True, stop=True
                    )
                # relu + f32->bf16 cast fused into the 4-bank PSUM evict
                st = stage_pool.tile([OUT, EV], BF16, tag="st", name="st")
                ev_eng.tensor_scalar_max(
                    st.rearrange("p (q c) -> p q c", q=EV // MM), acc[:], 0.0
                )
                nc.scalar.dma_start(
                    out=y.ap()[:, g * EV : (g + 1) * EV], in_=st[:]
                )

    split_multi_waits(nc)
    return nc


_nc_cache = {}

MODE = "pe"


def _get_program():
    if "nc" not in _nc_cache:
        _nc_cache["nc"] = (
            build_program_pe() if MODE == "pe" else build_program()
        )
    return _nc_cache["nc"]


def make_in_maps(x):
    """Full f32 x (B,W,L) -> per-core bf16 in_maps (layout depends on MODE)."""
    xb = np.ascontiguousarray(x).astype(NP_BF16)
    shards = xb.reshape(N_CORES, ROWS_PER_CORE, L)
    if MODE == "pe":
        mmat = conv_matrix()
        xT = np.ascontiguousarray(shards.transpose(0, 2, 1))  # [8, 128, n]
        return [{"x": xT[i], "m": mmat} for i in range(N_CORES)]
    return [{"x": shards[i]} for i in range(N_CORES)]


def kernel(x):
    x = np.asarray(x, dtype=np.float32)
    assert x.shape == (B, W, L), x.shape

    nc = _get_program()
    in_maps = make_in_maps(x)
    res = run_bass_kernel_spmd(nc, in_maps, core_ids=list(range(N_CORES)))
    if MODE == "pe":
        # y comes back [84, n] per core; transpose to letters-major
        out = np.concatenate(
            [
                np.asarray(res.results[i]["y"]).T.reshape(-1, W, OUT)
                for i in range(N_CORES)
            ],
            axis=0,
        )
    else:
        out = np.concatenate(
            [
                np.asarray(res.results[i]["y"]).reshape(-1, W, OUT)
                for i in range(N_CORES)
            ],
            axis=0,
        )
    return out.astype(np.float32)


# revision 8
# speedup vs baseline: 1.5383x; 1.1831x over previous
"""Trainium2 Bass kernel for nn_Conv_27693949125154.

Each 128-dim vector is a 16x8 image; valid 3x3 conv with the fixed kernel
[[1,0,1],[0,1,0],[1,0,1]] then relu -> 84 outputs (14x6).

All five stencil taps live inside each letter's own 128-element block
(flat pixel index k = i*8 + j): out_k = x[k] + x[k+2] + x[k+16] + x[k+18]
+ x[k+9].  That makes the first two partial sums expressible as *fully
contiguous* shifted adds over the whole chunk (full-rate DVE), with only
the final tap-gather op strided:

    h[k] = x[k] + x[k+2]        contiguous, 128*t-2 elems
    p[k] = h[k] + h[k+16]       contiguous, 128*t-18 elems
    s[t,i,j] = p[t,128t+8i+j] + x[t,128t+8i+j+9]   (strided, 84/letter)
    y = relu(s)                 ACT engine, contiguous, + store

The whole pipeline runs in bf16 (the conv is 4 adds; measured end-to-end
rel err ~7e-3, well inside the 2e-2 gate), which halves both HBM traffic
(12.2 MB/core -> ~34 us roofline at 358 GB/s) and DVE element time.
The host casts f32->bf16 before upload and back after.

Layout: letters (B*W rows) on SBUF partitions, pixels along the free
dim.  Input loads double-ring (sync + scalar HWDGE) with small leading
reads for pipeline ramp; relu'd outputs stream out on the scalar ring.

Pure data parallel over 8 NeuronCores (batch sharding, no comm).
"""

import numpy as np
import ml_dtypes

import concourse.bass as bass
import concourse.mybir as mybir
from concourse import tile
from concourse.bass_utils import run_bass_kernel_spmd

# Full problem: x (16384, 14, 128) f32 -> out (16384, 14, 84) f32
B, W, L = 16384, 14, 128
OUT = 84
N_CORES = 8
ROWS = B * W                     # 229376 letters total
ROWS_PER_CORE = ROWS // N_CORES  # 28672
P = 128                          # SBUF partitions

BF16 = mybir.dt.bfloat16
NP_BF16 = ml_dtypes.bfloat16


def split_multi_waits(nc, max_waits=1):
    """walrus CoreV3 codegen rejects instructions with several sync-wait
    conditions; hoist extras onto NOPs inserted just before, same engine."""
    for f in nc.m.functions:
        for blk in f.blocks:
            new = []
            for inst in blk.instructions:
                si = inst.sync_info
                if si is not None and si.on_wait and len(si.on_wait) > max_waits:
                    waits = list(si.on_wait)
                    head, tail = waits[:-max_waits], waits[-max_waits:]
                    for k, w in enumerate(head):
                        new.append(
                            mybir.InstNoOp(
                                name=f"{inst.name}-wsplit{k}",
                                engine=inst.engine,
                                ins=[],
                                outs=[],
                                sync_info=mybir.SyncInfo(on_wait=[w], on_update=[]),
                            )
                        )
                    inst.sync_info = mybir.SyncInfo(
                        on_wait=tail, on_update=list(si.on_update)
                    )
                new.append(inst)
            blk.instructions = new


def build_program(rows=ROWS_PER_CORE, read_sizes=None, chunk_sizes=None,
                  split_waits=True, o_bufs=2, work_bufs=2, r_bufs=2,
                  s_engine="vector"):
    """Per-core program: x [rows,128] bf16 -> y [rows,84] bf16."""
    t_total = rows // P                  # letters per partition (224)
    if read_sizes is None:
        read_sizes = [4, 4, 6, 14, 14, 14] + [28] * ((t_total - 56) // 28)
    if chunk_sizes is None:
        chunk_sizes = [7, 14, 42, 42, 42, 42, 21, 7, 7]
    assert sum(read_sizes) == t_total and sum(chunk_sizes) == t_total
    t_c_max = max(chunk_sizes)

    nc = bass.Bass(
        "TRN2", target_bir_lowering=False, debug=False, num_devices=N_CORES
    )
    x = nc.dram_tensor("x", [rows, L], BF16, kind="ExternalInput")
    y = nc.dram_tensor("y", [rows, OUT], BF16, kind="ExternalOutput")

    # partition p holds letters [p*t_total, (p+1)*t_total)
    xf = x.ap().rearrange("(p t) m -> p (t m)", p=P)   # [P, t_total*128]
    yf = y.ap().rearrange("(p t) m -> p (t m)", p=P)   # [P, t_total*84]

    with tile.TileContext(nc) as tc:
        with (
            tc.tile_pool(name="xin", bufs=1) as xin_pool,
            tc.tile_pool(name="oout", bufs=o_bufs) as oout_pool,
            tc.tile_pool(name="hpool", bufs=r_bufs) as hpool,
            tc.tile_pool(name="ppool", bufs=r_bufs) as ppool,
            tc.tile_pool(name="spool", bufs=work_bufs) as spool,
        ):
            xt = xin_pool.tile([P, t_total * L], BF16, tag="x")
            # all reads upfront into disjoint slices -> max read-ahead
            off = 0
            for k, sz in enumerate(read_sizes):
                eng = nc.scalar if (k % 2 == 1 and k < 8) else nc.sync
                eng.dma_start(
                    out=xt[:, off * L : (off + sz) * L],
                    in_=xf[:, off * L : (off + sz) * L],
                )
                off += sz

            X4 = xt.rearrange("p (t i j) -> p t i j", i=16, j=8)  # [P,t,16,8]
            s_eng = nc.gpsimd if s_engine == "gpsimd" else nc.vector
            off = 0
            for t_c in chunk_sizes:
                x0 = off * L                       # chunk base, flat elems
                n1 = t_c * L - 2
                n2 = t_c * L - 18

                # h[k] = x[k] + x[k+2], contiguous full-rate
                ht = hpool.tile([P, t_c_max * L], BF16, tag="h", name="h")
                nc.vector.tensor_tensor(
                    ht[:, :n1], xt[:, x0 : x0 + n1], xt[:, x0 + 2 : x0 + 2 + n1],
                    mybir.AluOpType.add,
                )

                # p[k] = h[k] + h[k+16], contiguous full-rate
                pt = ppool.tile([P, t_c_max * L], BF16, tag="p", name="p")
                nc.vector.tensor_tensor(
                    pt[:, :n2], ht[:, :n2], ht[:, 16 : 16 + n2],
                    mybir.AluOpType.add,
                )

                # s = p(taps) + x(center taps), strided gather -> compact 84
                st = spool.tile([P, t_c_max * 84], BF16, tag="s", name="s")
                s4 = st.rearrange("p (t i j) -> p t i j", i=14, j=6)[:, :t_c]
                p4 = pt.rearrange("p (t i j) -> p t i j", i=16, j=8)
                s_eng.tensor_tensor(
                    s4, p4[:, :t_c, 0:14, 0:6], X4[:, off : off + t_c, 1:15, 1:7],
                    mybir.AluOpType.add,
                )

                # relu on the scalar engine; out-DMA on the scalar ring
                ot = oout_pool.tile([P, t_c_max * OUT], BF16, tag="o", name="ot")[
                    :, : t_c * OUT
                ]
                nc.scalar.activation(
                    ot[:], st[:, : t_c * 84], mybir.ActivationFunctionType.Relu
                )
                nc.scalar.dma_start(
                    out=yf[:, off * OUT : (off + t_c) * OUT], in_=ot[:]
                )
                off += t_c

    if split_waits:
        split_multi_waits(nc)
    return nc


N_LET = ROWS_PER_CORE                 # letters per core (28672)
MM = 512                              # letters per matmul (PSUM bank = 512 f32)
EV = 2048                             # letters per evict / store chunk (4 banks)


def conv_matrix():
    """[128, 84] 0/1 tap matrix: out[:, o] = sum of 5 taps of the letter."""
    m = np.zeros((L, OUT), dtype=np.float32)
    for i in range(14):
        for j in range(6):
            o = i * 6 + j
            for di, dj in ((0, 0), (0, 2), (1, 1), (2, 0), (2, 2)):
                m[(i + di) * 8 + (j + dj), o] = 1.0
    return m.astype(NP_BF16)


def build_program_pe(n=N_LET, read_sizes=None, groups=None, n_scalar_stores=4):
    """PE-matmul program: xT [128, n] bf16 -> y [84, n] bf16.

    Host supplies x transposed (pixels on partitions).  One matmul per 512
    letters against the 0/1 tap matrix does all 4 adds with exact f32 PSUM
    accumulation; relu + bf16 cast fused into the PSUM evict, alternating
    DVE / ACT (both run it at ~1 elem/cycle; splitting halves the tax).
    Early stores ride the scalar ring (sync still busy with reads); late
    stores the sync ring.
    """
    if read_sizes is None:
        read_sizes = [512, 512, 1024, 2048, 2048, 2048] + [4096] * 5
    if groups is None:
        groups = [1024, 1024] + [2048] * 12 + [1024, 512, 512]
    assert sum(read_sizes) == n and sum(groups) == n
    assert all(g % MM == 0 or g < 2048 for g in groups)

    nc = bass.Bass(
        "TRN2", target_bir_lowering=False, debug=False, num_devices=N_CORES
    )
    x = nc.dram_tensor("x", [L, n], BF16, kind="ExternalInput")
    m = nc.dram_tensor("m", [L, OUT], BF16, kind="ExternalInput")
    y = nc.dram_tensor("y", [OUT, n], BF16, kind="ExternalOutput")

    with tile.TileContext(nc) as tc:
        with (
            tc.tile_pool(name="xin", bufs=1) as xin_pool,
            tc.tile_pool(name="mw", bufs=1) as mw_pool,
            tc.tile_pool(name="stage", bufs=3) as stage_pool,
            tc.psum_pool(name="acc", bufs=2) as acc_pool,
        ):
            mt = mw_pool.tile([L, OUT], BF16, tag="m")
            nc.sync.dma_start(out=mt[:], in_=m.ap())

            xt = xin_pool.tile([L, n], BF16, tag="x")
            off = 0
            for k, sz in enumerate(read_sizes):
                eng = nc.scalar if (k % 2 == 1 and k < 6) else nc.sync
                eng.dma_start(out=xt[:, off : off + sz], in_=x.ap()[:, off : off + sz])
                off += sz

            off = 0
            for g, ev in enumerate(groups):
                nmm = ev // MM
                assert nmm * MM == ev
                acc = acc_pool.tile(
                    [OUT, EV // MM, MM], mybir.dt.float32, tag="acc", name="acc"
                )
                for q in range(nmm):
                    a = off + q * MM
                    nc.tensor.matmul(
                        acc[:, q], mt[:], xt[:, a : a + MM],
                        start=True, stop=True,
                    )
                # relu + f32->bf16 cast fused into the multi-bank PSUM evict,
                # alternating DVE / ACT to split the 1x-mode evict tax
                st = stage_pool.tile([OUT, EV], BF16, tag="st", name="st")
                sv = st[:, :ev].rearrange("p (q c) -> p q c", q=nmm)
                av = acc[:, :nmm]
                if g % 2 == 0:
                    nc.vector.tensor_scalar_max(sv, av, 0.0)
                else:
                    nc.scalar.activation(
                        sv, av, mybir.ActivationFunctionType.Relu
                    )
                st_eng = nc.scalar if g < n_scalar_stores else nc.sync
                st_eng.dma_start(out=y.ap()[:, off : off + ev], in_=st[:, :ev])
                off += ev

    split_multi_waits(nc)
    return nc


_nc_cache = {}

MODE = "pe"


def _get_program():
    if "nc" not in _nc_cache:
        _nc_cache["nc"] = (
            build_program_pe() if MODE == "pe" else build_program()
        )
    return _nc_cache["nc"]


def make_in_maps(x):
    """Full f32 x (B,W,L) -> per-core bf16 in_maps (layout depends on MODE)."""
    xb = np.ascontiguousarray(x).astype(NP_BF16)
    shards = xb.reshape(N_CORES, ROWS_PER_CORE, L)
    if MODE == "pe":
        mmat = conv_matrix()
        xT = np.ascontiguousarray(shards.transpose(0, 2, 1))  # [8, 128, n]
        return [{"x": xT[i], "m": mmat} for i in range(N_CORES)]
    return [{"x": shards[i]} for i in range(N_CORES)]


def kernel(x):
    x = np.asarray(x, dtype=np.float32)
    assert x.shape == (B, W, L), x.shape

    nc = _get_program()
    in_maps = make_in_maps(x)
    res = run_bass_kernel_spmd(nc, in_maps, core_ids=list(range(N_CORES)))
    if MODE == "pe":
        # y comes back [84, n] per core; transpose to letters-major
        out = np.concatenate(
            [
                np.asarray(res.results[i]["y"]).T.reshape(-1, W, OUT)
                for i in range(N_CORES)
            ],
            axis=0,
        )
    else:
        out = np.concatenate(
            [
                np.asarray(res.results[i]["y"]).reshape(-1, W, OUT)
                for i in range(N_CORES)
            ],
            axis=0,
        )
    return out.astype(np.float32)


# revision 12
# speedup vs baseline: 1.7208x; 1.1186x over previous
"""Trainium2 Bass kernel for nn_Conv_27693949125154.

Each 128-dim vector is a 16x8 image; valid 3x3 conv with the fixed kernel
[[1,0,1],[0,1,0],[1,0,1]] then relu -> 84 outputs (14x6).

All five stencil taps live inside each letter's own 128-element block
(flat pixel index k = i*8 + j): out_k = x[k] + x[k+2] + x[k+16] + x[k+18]
+ x[k+9].  That makes the first two partial sums expressible as *fully
contiguous* shifted adds over the whole chunk (full-rate DVE), with only
the final tap-gather op strided:

    h[k] = x[k] + x[k+2]        contiguous, 128*t-2 elems
    p[k] = h[k] + h[k+16]       contiguous, 128*t-18 elems
    s[t,i,j] = p[t,128t+8i+j] + x[t,128t+8i+j+9]   (strided, 84/letter)
    y = relu(s)                 ACT engine, contiguous, + store

The whole pipeline runs in bf16 (the conv is 4 adds; measured end-to-end
rel err ~7e-3, well inside the 2e-2 gate), which halves both HBM traffic
(12.2 MB/core -> ~34 us roofline at 358 GB/s) and DVE element time.
The host casts f32->bf16 before upload and back after.

Layout: letters (B*W rows) on SBUF partitions, pixels along the free
dim.  Input loads double-ring (sync + scalar HWDGE) with small leading
reads for pipeline ramp; relu'd outputs stream out on the scalar ring.

Pure data parallel over 8 NeuronCores (batch sharding, no comm).
"""

import numpy as np
import ml_dtypes

import concourse.bass as bass
import concourse.mybir as mybir
from concourse import tile
from concourse.bass_utils import run_bass_kernel_spmd

# Full problem: x (16384, 14, 128) f32 -> out (16384, 14, 84) f32
B, W, L = 16384, 14, 128
OUT = 84
N_CORES = 8
ROWS = B * W                     # 229376 letters total
ROWS_PER_CORE = ROWS // N_CORES  # 28672
P = 128                          # SBUF partitions

BF16 = mybir.dt.bfloat16
NP_BF16 = ml_dtypes.bfloat16


def split_multi_waits(nc, max_waits=1):
    """walrus CoreV3 codegen rejects instructions with several sync-wait
    conditions; hoist extras onto NOPs inserted just before, same engine."""
    for f in nc.m.functions:
        for blk in f.blocks:
            new = []
            for inst in blk.instructions:
                si = inst.sync_info
                if si is not None and si.on_wait and len(si.on_wait) > max_waits:
                    waits = list(si.on_wait)
                    head, tail = waits[:-max_waits], waits[-max_waits:]
                    for k, w in enumerate(head):
                        new.append(
                            mybir.InstNoOp(
                                name=f"{inst.name}-wsplit{k}",
                                engine=inst.engine,
                                ins=[],
                                outs=[],
                                sync_info=mybir.SyncInfo(on_wait=[w], on_update=[]),
                            )
                        )
                    inst.sync_info = mybir.SyncInfo(
                        on_wait=tail, on_update=list(si.on_update)
                    )
                new.append(inst)
            blk.instructions = new


def build_program(rows=ROWS_PER_CORE, read_sizes=None, chunk_sizes=None,
                  split_waits=True, o_bufs=2, work_bufs=2, r_bufs=2,
                  s_engine="vector"):
    """Per-core program: x [rows,128] bf16 -> y [rows,84] bf16."""
    t_total = rows // P                  # letters per partition (224)
    if read_sizes is None:
        read_sizes = [4, 4, 6, 14, 14, 14] + [28] * ((t_total - 56) // 28)
    if chunk_sizes is None:
        chunk_sizes = [7, 14, 42, 42, 42, 42, 21, 7, 7]
    assert sum(read_sizes) == t_total and sum(chunk_sizes) == t_total
    t_c_max = max(chunk_sizes)

    nc = bass.Bass(
        "TRN2", target_bir_lowering=False, debug=False, num_devices=N_CORES
    )
    x = nc.dram_tensor("x", [rows, L], BF16, kind="ExternalInput")
    y = nc.dram_tensor("y", [rows, OUT], BF16, kind="ExternalOutput")

    # partition p holds letters [p*t_total, (p+1)*t_total)
    xf = x.ap().rearrange("(p t) m -> p (t m)", p=P)   # [P, t_total*128]
    yf = y.ap().rearrange("(p t) m -> p (t m)", p=P)   # [P, t_total*84]

    with tile.TileContext(nc) as tc:
        with (
            tc.tile_pool(name="xin", bufs=1) as xin_pool,
            tc.tile_pool(name="oout", bufs=o_bufs) as oout_pool,
            tc.tile_pool(name="hpool", bufs=r_bufs) as hpool,
            tc.tile_pool(name="ppool", bufs=r_bufs) as ppool,
            tc.tile_pool(name="spool", bufs=work_bufs) as spool,
        ):
            xt = xin_pool.tile([P, t_total * L], BF16, tag="x")
            # all reads upfront into disjoint slices -> max read-ahead
            off = 0
            for k, sz in enumerate(read_sizes):
                eng = nc.scalar if (k % 2 == 1 and k < 8) else nc.sync
                eng.dma_start(
                    out=xt[:, off * L : (off + sz) * L],
                    in_=xf[:, off * L : (off + sz) * L],
                )
                off += sz

            X4 = xt.rearrange("p (t i j) -> p t i j", i=16, j=8)  # [P,t,16,8]
            s_eng = nc.gpsimd if s_engine == "gpsimd" else nc.vector
            off = 0
            for t_c in chunk_sizes:
                x0 = off * L                       # chunk base, flat elems
                n1 = t_c * L - 2
                n2 = t_c * L - 18

                # h[k] = x[k] + x[k+2], contiguous full-rate
                ht = hpool.tile([P, t_c_max * L], BF16, tag="h", name="h")
                nc.vector.tensor_tensor(
                    ht[:, :n1], xt[:, x0 : x0 + n1], xt[:, x0 + 2 : x0 + 2 + n1],
                    mybir.AluOpType.add,
                )

                # p[k] = h[k] + h[k+16], contiguous full-rate
                pt = ppool.tile([P, t_c_max * L], BF16, tag="p", name="p")
                nc.vector.tensor_tensor(
                    pt[:, :n2], ht[:, :n2], ht[:, 16 : 16 + n2],
                    mybir.AluOpType.add,
                )

                # s = p(taps) + x(center taps), strided gather -> compact 84
                st = spool.tile([P, t_c_max * 84], BF16, tag="s", name="s")
                s4 = st.rearrange("p (t i j) -> p t i j", i=14, j=6)[:, :t_c]
                p4 = pt.rearrange("p (t i j) -> p t i j", i=16, j=8)
                s_eng.tensor_tensor(
                    s4, p4[:, :t_c, 0:14, 0:6], X4[:, off : off + t_c, 1:15, 1:7],
                    mybir.AluOpType.add,
                )

                # relu on the scalar engine; out-DMA on the scalar ring
                ot = oout_pool.tile([P, t_c_max * OUT], BF16, tag="o", name="ot")[
                    :, : t_c * OUT
                ]
                nc.scalar.activation(
                    ot[:], st[:, : t_c * 84], mybir.ActivationFunctionType.Relu
                )
                nc.scalar.dma_start(
                    out=yf[:, off * OUT : (off + t_c) * OUT], in_=ot[:]
                )
                off += t_c

    if split_waits:
        split_multi_waits(nc)
    return nc


N_LET = ROWS_PER_CORE                 # letters per core (28672)
MM = 512                              # letters per matmul (PSUM bank = 512 f32)
EV = 2048                             # letters per evict / store chunk (4 banks)


def conv_matrix():
    """[128, 84] 0/1 tap matrix: out[:, o] = sum of 5 taps of the letter."""
    m = np.zeros((L, OUT), dtype=np.float32)
    for i in range(14):
        for j in range(6):
            o = i * 6 + j
            for di, dj in ((0, 0), (0, 2), (1, 1), (2, 0), (2, 2)):
                m[(i + di) * 8 + (j + dj), o] = 1.0
    return m.astype(NP_BF16)


def build_program_pe(n=N_LET, read_sizes=None, groups=None, n_scalar_stores=4):
    """PE-matmul program: xT [128, n] bf16 -> y [84, n] bf16.

    Host supplies x transposed (pixels on partitions).  One matmul per 512
    letters against the 0/1 tap matrix does all 4 adds with exact f32 PSUM
    accumulation; relu + bf16 cast fused into the PSUM evict, alternating
    DVE / ACT (both run it at ~1 elem/cycle; splitting halves the tax).
    Early stores ride the scalar ring (sync still busy with reads); late
    stores the sync ring.
    """
    if read_sizes is None:
        read_sizes = [512, 512, 1024, 2048, 2048, 2048] + [4096] * 5
    if groups is None:
        groups = [1024, 1024] + [2048] * 12 + [1024, 512, 512]
    assert sum(read_sizes) == n and sum(groups) == n
    assert all(g % MM == 0 or g < 2048 for g in groups)

    nc = bass.Bass(
        "TRN2", target_bir_lowering=False, debug=False, num_devices=N_CORES
    )
    x = nc.dram_tensor("x", [L, n], BF16, kind="ExternalInput")
    m = nc.dram_tensor("m", [L, OUT], BF16, kind="ExternalInput")
    y = nc.dram_tensor("y", [OUT, n], BF16, kind="ExternalOutput")

    with tile.TileContext(nc) as tc:
        with (
            tc.tile_pool(name="xin", bufs=1) as xin_pool,
            tc.tile_pool(name="mw", bufs=1) as mw_pool,
            tc.tile_pool(name="stage", bufs=3) as stage_pool,
            tc.psum_pool(name="acc", bufs=2) as acc_pool,
        ):
            mt = mw_pool.tile([L, OUT], BF16, tag="m")
            nc.sync.dma_start(out=mt[:], in_=m.ap())

            xt = xin_pool.tile([L, n], BF16, tag="x")
            off = 0
            for k, sz in enumerate(read_sizes):
                eng = nc.scalar if (k % 2 == 1 and k < 6) else nc.sync
                eng.dma_start(out=xt[:, off : off + sz], in_=x.ap()[:, off : off + sz])
                off += sz

            off = 0
            for g, ev in enumerate(groups):
                nmm = ev // MM
                assert nmm * MM == ev
                acc = acc_pool.tile(
                    [OUT, EV // MM, MM], mybir.dt.float32, tag="acc", name="acc"
                )
                for q in range(nmm):
                    a = off + q * MM
                    nc.tensor.matmul(
                        acc[:, q], mt[:], xt[:, a : a + MM],
                        start=True, stop=True,
                    )
                # relu + f32->bf16 cast fused into the multi-bank PSUM evict,
                # alternating DVE / ACT to split the 1x-mode evict tax
                st = stage_pool.tile([OUT, EV], BF16, tag="st", name="st")
                sv = st[:, :ev].rearrange("p (q c) -> p q c", q=nmm)
                av = acc[:, :nmm]
                if g % 2 == 0:
                    nc.vector.tensor_scalar_max(sv, av, 0.0)
                else:
                    nc.scalar.activation(
                        sv, av, mybir.ActivationFunctionType.Relu
                    )
                st_eng = nc.scalar if g < n_scalar_stores else nc.sync
                st_eng.dma_start(out=y.ap()[:, off : off + ev], in_=st[:, :ev])
                off += ev

    split_multi_waits(nc)
    return nc


A_PE = 16384                          # letters on the PE-matmul path
B_DVE = ROWS_PER_CORE - A_PE          # letters on the DVE-adds path (12288)
T_B = B_DVE // P                      # per-partition letters, DVE path (96)


def build_program_hybrid(
    pe_groups=None, pe_reads=None, dve_chunks=None, dve_reads=None,
    n_scalar_stores=2,
):
    """Hybrid: PE-matmul path for A_PE letters (pixel-major xT [128, A]) in
    parallel with the contiguous-adds DVE path for B_DVE letters (letter-major
    xr [B, 128]).  Both paths bf16.  Engine budget: PE ~21us matmul, DVE
    ~21us (adds + half the PE evicts), ACT ~27us (relu + other evicts +
    stores), all under the ~34us DMA roofline for 12.2 MB.
    """
    if pe_groups is None:
        pe_groups = [1024, 1024] + [2048] * 6 + [1024, 512, 512]
    if pe_reads is None:
        pe_reads = [1024, 1024, 2048, 4096, 4096, 4096]
    if dve_chunks is None:
        dve_chunks = [8, 16, 24, 24, 16, 8]
    if dve_reads is None:
        dve_reads = [16, 32, 48]
    assert sum(pe_groups) == A_PE and sum(pe_reads) == A_PE
    assert sum(dve_chunks) == T_B and sum(dve_reads) == T_B
    t_c_max = max(dve_chunks)

    nc = bass.Bass(
        "TRN2", target_bir_lowering=False, debug=False, num_devices=N_CORES
    )
    xT = nc.dram_tensor("xT", [L, A_PE], BF16, kind="ExternalInput")
    xr = nc.dram_tensor("xr", [B_DVE, L], BF16, kind="ExternalInput")
    m = nc.dram_tensor("m", [L, OUT], BF16, kind="ExternalInput")
    y1 = nc.dram_tensor("y1", [OUT, A_PE], BF16, kind="ExternalOutput")
    y2 = nc.dram_tensor("y2", [B_DVE, OUT], BF16, kind="ExternalOutput")

    xrf = xr.ap().rearrange("(p t) m -> p (t m)", p=P)   # [P, T_B*128]
    y2f = y2.ap().rearrange("(p t) m -> p (t m)", p=P)   # [P, T_B*84]

    with tile.TileContext(nc) as tc:
        with (
            tc.tile_pool(name="xTin", bufs=1) as xT_pool,
            tc.tile_pool(name="xrin", bufs=1) as xr_pool,
            tc.tile_pool(name="mw", bufs=1) as mw_pool,
            tc.tile_pool(name="stage", bufs=3) as stage_pool,
            tc.tile_pool(name="hpool", bufs=2) as hpool,
            tc.tile_pool(name="ppool", bufs=2) as ppool,
            tc.tile_pool(name="spool", bufs=2) as spool,
            tc.tile_pool(name="opool", bufs=2) as opool,
            tc.psum_pool(name="acc", bufs=2) as acc_pool,
        ):
            mt = mw_pool.tile([L, OUT], BF16, tag="m")
            nc.sync.dma_start(out=mt[:], in_=m.ap())

            xTt = xT_pool.tile([L, A_PE], BF16, tag="xT")
            off = 0
            for sz in pe_reads:
                nc.sync.dma_start(
                    out=xTt[:, off : off + sz], in_=xT.ap()[:, off : off + sz]
                )
                off += sz
            xrt = xr_pool.tile([P, T_B * L], BF16, tag="xr")
            off = 0
            for sz in dve_reads:
                nc.scalar.dma_start(
                    out=xrt[:, off * L : (off + sz) * L],
                    in_=xrf[:, off * L : (off + sz) * L],
                )
                off += sz

            XR4 = xrt.rearrange("p (t i j) -> p t i j", i=16, j=8)

            def pe_group(g, off, ev):
                nmm = ev // MM
                acc = acc_pool.tile(
                    [OUT, EV // MM, MM], mybir.dt.float32, tag="acc", name="acc"
                )
                for q in range(nmm):
                    nc.tensor.matmul(
                        acc[:, q], mt[:], xTt[:, off + q * MM : off + (q + 1) * MM],
                        start=True, stop=True,
                    )
                st = stage_pool.tile([OUT, EV], BF16, tag="st", name="st")
                sv = st[:, :ev].rearrange("p (q c) -> p q c", q=nmm)
                av = acc[:, :nmm]
                if g % 2 == 0:
                    nc.vector.tensor_scalar_max(sv, av, 0.0)
                else:
                    nc.scalar.activation(
                        sv, av, mybir.ActivationFunctionType.Relu
                    )
                st_eng = nc.scalar if g < n_scalar_stores else nc.sync
                st_eng.dma_start(out=y1.ap()[:, off : off + ev], in_=st[:, :ev])

            def dve_chunk(off, t_c):
                x0 = off * L
                n1 = t_c * L - 2
                n2 = t_c * L - 18
                ht = hpool.tile([P, t_c_max * L], BF16, tag="h", name="h")
                nc.vector.tensor_tensor(
                    ht[:, :n1], xrt[:, x0 : x0 + n1],
                    xrt[:, x0 + 2 : x0 + 2 + n1], mybir.AluOpType.add,
                )
                pt = ppool.tile([P, t_c_max * L], BF16, tag="p", name="p")
                nc.vector.tensor_tensor(
                    pt[:, :n2], ht[:, :n2], ht[:, 16 : 16 + n2],
                    mybir.AluOpType.add,
                )
                st = spool.tile([P, t_c_max * 84], BF16, tag="s", name="s")
                s4 = st.rearrange("p (t i j) -> p t i j", i=14, j=6)[:, :t_c]
                p4 = pt.rearrange("p (t i j) -> p t i j", i=16, j=8)
                nc.vector.tensor_tensor(
                    s4, p4[:, :t_c, 0:14, 0:6],
                    XR4[:, off : off + t_c, 1:15, 1:7], mybir.AluOpType.add,
                )
                ot = opool.tile([P, t_c_max * OUT], BF16, tag="o", name="ot")[
                    :, : t_c * OUT
                ]
                nc.scalar.activation(
                    ot[:], st[:, : t_c * 84], mybir.ActivationFunctionType.Relu
                )
                nc.scalar.dma_start(
                    out=y2f[:, off * OUT : (off + t_c) * OUT], in_=ot[:]
                )

            # interleave the two paths so both engine streams stay fed
            pe_offs = np.cumsum([0] + pe_groups[:-1]).tolist()
            dve_offs = np.cumsum([0] + dve_chunks[:-1]).tolist()
            ng, ndc = len(pe_groups), len(dve_chunks)
            di = 0
            for g in range(ng):
                pe_group(g, pe_offs[g], pe_groups[g])
                # spread DVE chunks between PE groups
                while di < ndc and (di + 1) * ng <= (g + 1) * ndc:
                    dve_chunk(dve_offs[di], dve_chunks[di])
                    di += 1
            while di < ndc:
                dve_chunk(dve_offs[di], dve_chunks[di])
                di += 1

    split_multi_waits(nc)
    return nc


_nc_cache = {}

MODE = "hybrid"


def _get_program():
    if "nc" not in _nc_cache:
        _nc_cache["nc"] = {
            "pe": build_program_pe,
            "hybrid": build_program_hybrid,
            "dve": build_program,
        }[MODE]()
    return _nc_cache["nc"]


def make_in_maps(x):
    """Full f32 x (B,W,L) -> per-core bf16 in_maps (layout depends on MODE)."""
    xb = np.ascontiguousarray(x).astype(NP_BF16)
    shards = xb.reshape(N_CORES, ROWS_PER_CORE, L)
    if MODE == "pe":
        mmat = conv_matrix()
        xT = np.ascontiguousarray(shards.transpose(0, 2, 1))  # [8, 128, n]
        return [{"x": xT[i], "m": mmat} for i in range(N_CORES)]
    if MODE == "hybrid":
        mmat = conv_matrix()
        xT = np.ascontiguousarray(shards[:, :A_PE].transpose(0, 2, 1))
        xr = np.ascontiguousarray(shards[:, A_PE:])
        return [{"xT": xT[i], "xr": xr[i], "m": mmat} for i in range(N_CORES)]
    return [{"x": shards[i]} for i in range(N_CORES)]


def kernel(x):
    x = np.asarray(x, dtype=np.float32)
    assert x.shape == (B, W, L), x.shape

    nc = _get_program()
    in_maps = make_in_maps(x)
    res = run_bass_kernel_spmd(nc, in_maps, core_ids=list(range(N_CORES)))
    if MODE == "pe":
        # y comes back [84, n] per core; transpose to letters-major
        out = np.concatenate(
            [
                np.asarray(res.results[i]["y"]).T.reshape(-1, W, OUT)
                for i in range(N_CORES)
            ],
            axis=0,
        )
    elif MODE == "hybrid":
        parts = []
        for i in range(N_CORES):
            a = np.asarray(res.results[i]["y1"]).T      # [A_PE, 84]
            b_ = np.asarray(res.results[i]["y2"])       # [B_DVE, 84]
            parts.append(np.concatenate([a, b_], axis=0).reshape(-1, W, OUT))
        out = np.concatenate(parts, axis=0)
    else:
        out = np.concatenate(
            [
                np.asarray(res.results[i]["y"]).reshape(-1, W, OUT)
                for i in range(N_CORES)
            ],
            axis=0,
        )
    return out.astype(np.float32)
